# revision 13
# baseline (speedup 1.0000x reference)
"""Causal multi-head attention on 8 Trainium2 NeuronCores.

Sharding: core c handles batch b = c//2 and head-half hg = c%2 (8 of 16
heads, as 4 pairs). Per core: QKV projection (bf16 matmuls, f32 PSUM),
flash-style causal attention in transposed layout (scores_T[t, s], softmax
denominator via a ones-column at v-col 0), per-pair pairwise AllGather of
the normalized attention outputs, and a column-parallel output projection
(w_o columns sharded host-side per core parity). Host reassembles y from
the per-core [m_half, s] transposed outputs.

Loop structure: s-tiles outer; each s-tile's attention is interleaved with
the next s-tile's QKV-projection matmuls and the previous s-tile's output
projection so TensorE stays dense while ScalarE runs the exps; each pair's
AllGather fires right after its normalize so transfers hide under later
pairs' attention. The last s-tile's output projection accumulates per-pair
into an SBUF f32 tile so only 2 chunks x 4 mt matmuls remain after the
final AllGather lands.
"""
import sys

sys.path.insert(0, "/opt/trn_rl_repo")

import numpy as np
import ml_dtypes

import concourse.bass as bass
import concourse.mybir as mybir
import concourse.tile as tile
from concourse import bacc
from concourse.bass_utils import run_bass_kernel_spmd

BF16 = ml_dtypes.bfloat16
DT = mybir.dt.bfloat16
F32 = mybir.dt.float32
EXP = mybir.ActivationFunctionType.Exp

B, S, DM, H, DK = 4, 2048, 1024, 16, 64
N_CORES = 8
N_PAIRS = 4          # head pairs per core (8 heads)
N_MCH = DM // 128    # m-chunks of the model dim (contraction for QKV proj)
REPLICA_GROUPS = [[0, 1], [2, 3], [4, 5], [6, 7]]


def build_nc(seq=S, n_pairs=N_PAIRS):
    """Build the SPMD kernel graph. seq must be a multiple of 512."""
    nst = seq // 512          # 512-wide s-tiles
    ntt_all = seq // 128      # 128-wide t-tiles
    nc = bacc.Bacc("TRN2", target_bir_lowering=False, debug=False,
                   num_devices=N_CORES)

    nst_ = seq // 512
    xT = nc.dram_tensor("xT", [128, nst_, N_MCH, 512], DT,
                        kind="ExternalInput")
    wq = nc.dram_tensor("wq", [128, N_MCH, 128 * n_pairs], DT,
                        kind="ExternalInput")
    wk = nc.dram_tensor("wk", [128, N_MCH, 128 * n_pairs], DT,
                        kind="ExternalInput")
    wv = nc.dram_tensor("wv", [128, N_MCH, 128 * n_pairs], DT,
                        kind="ExternalInput")
    wo = nc.dram_tensor("wo", [128, 2 * n_pairs, 512], DT,
                        kind="ExternalInput")
    mask128 = nc.dram_tensor("mask128", [128, 128], DT, kind="ExternalInput")
    yT = nc.dram_tensor("yT", [512, seq], F32, kind="ExternalOutput")

    n_dch = 2 * n_pairs   # d-chunks of 128 in the gathered attention
    hw = 128 * n_pairs    # head-dim columns per core (2*n_pairs heads x 64)

    with tile.TileContext(nc) as tc:
        with (
            tc.tile_pool(name="dram", bufs=1, space="DRAM") as dram,
            tc.tile_pool(name="persist", bufs=1) as persist,
            tc.tile_pool(name="psum_p", bufs=1, space="PSUM") as pp,
            tc.tile_pool(name="psum_s", bufs=2, space="PSUM") as ps_s,
            tc.tile_pool(name="psum_av", bufs=3, space="PSUM") as ps_av,
            tc.tile_pool(name="pt", bufs=4) as p_pool,
            tc.tile_pool(name="nrm", bufs=2) as nrm,
            tc.tile_pool(name="yc", bufs=2) as ycp,
            tc.tile_pool(name="stg", bufs=3) as stg,
        ):
            # ag staging in the stage tile's own [p, h, s] layout so
            # every hop is a contiguous memcpy; af/wo use a matching
            # k-major (p' = k*2 + h) within-chunk row order (host-side)
            ag_in = dram.tile([nst, n_pairs, 64, 2, 512], DT)
            ag_out = dram.tile([nst, 2, n_pairs, 64, 2, 512], DT)
            ag_in3 = dram.tile([n_pairs, 64, 2, 512], DT)
            ag_out3 = dram.tile([n_pairs, 2, 64, 2, 512], DT)

            q_sb = persist.tile([128, n_pairs, seq], DT, tag="q")
            k_sb = persist.tile([128, n_pairs, seq], DT, tag="k")
            v_sb = persist.tile([128, ntt_all, 2 * n_pairs, 65], DT, tag="v")
            af_sb = persist.tile([128, n_dch, seq], DT, tag="af")
            m_sb = persist.tile([128, 128], DT, tag="m")
            wo_sb = persist.tile([128, n_dch, 512], DT, tag="wo")
            wq_sb = persist.tile([128, N_MCH, hw], DT, tag="wq")
            wk_sb = persist.tile([128, N_MCH, hw], DT, tag="wk")
            wv_sb = persist.tile([128, N_MCH, hw], DT, tag="wv")
            yacc = persist.tile([128, 4, 512], F32, tag="yacc")
            xt = []
            for st in range(nst):
                t = persist.tile([128, N_MCH, 512], DT, tag=f"xt{st}")
                xt.append(t)

            # spread input loads across per-engine DMA queues so the
            # critical tiles (xt0, wv, wq) land concurrently; host supplies
            # layouts that DMA as contiguous multi-KB descriptors
            nc.sync.dma_start(out=xt[0][:], in_=xT[:, 0])
            nc.scalar.dma_start(out=wv_sb[:], in_=wv[:])
            nc.gpsimd.dma_start(out=wq_sb[:], in_=wq[:])
            nc.scalar.dma_start(out=wk_sb[:], in_=wk[:])
            if nst > 1:
                nc.gpsimd.dma_start(out=xt[1][:], in_=xT[:, 1])
            for st in range(2, nst):
                nc.sync.dma_start(out=xt[st][:], in_=xT[:, st])
            nc.gpsimd.dma_start(out=m_sb[:], in_=mask128[:])
            nc.sync.dma_start(out=wo_sb[:], in_=wo[:])
            nc.vector.memset(v_sb[:, :, :, 64], 1.0)
            # PE warm-up during the input-load window: dummy matmuls on a
            # memset tile keep the HAM busy-window open (no DMA dependency)
            warm = persist.tile([128, 512], DT, tag="warm")
            nc.vector.memset(warm[:], 0.0)
            for wi in range(3):
                wps = ps_s.tile([128, 2, 512], F32, tag="sc",
                                name=f"warm{wi}")
                for wj in range(8):
                    nc.tensor.matmul(
                        wps[:, wj % 2, :],
                        lhsT=warm[:, 0:128], rhs=warm[:],
                        start=True, stop=True)

            yT_v = yT[:].rearrange("(t p) s -> p t s", p=128)

            # ---- emission helpers (each returns a closure doing one
            # PE-dense psum-group; used to fill PE during attention) ----
            def vproj_group(tt):
                def go():
                    st, r = tt // 4, tt % 4
                    ps = pp.tile([128, hw], F32, tag="proj", name=f"psv{tt}")
                    for c in range(N_MCH):
                        nc.tensor.matmul(
                            ps[:],
                            lhsT=xt[st][:, c, r * 128:(r + 1) * 128],
                            rhs=wv_sb[:, c, 0:hw],
                            start=(c == 0), stop=(c == N_MCH - 1))
                    nc.vector.tensor_copy(
                        v_sb[:, tt, :, 0:64],
                        ps[:].rearrange("p (h k) -> p h k", k=64))
                return go

            def qkproj_group(pair, st, which):
                def go():
                    w_sb, dst = ((wq_sb, q_sb), (wk_sb, k_sb))[which]
                    ps = pp.tile([128, 512], F32, tag="proj",
                                 name=f"psqk{pair}_{st}_{which}")
                    for c in range(N_MCH):
                        nc.tensor.matmul(
                            ps[:],
                            lhsT=w_sb[:, c, pair * 128:(pair + 1) * 128],
                            rhs=xt[st][:, c, :],
                            start=(c == 0), stop=(c == N_MCH - 1))
                    nc.vector.tensor_copy(
                        dst[:, pair, st * 512:(st + 1) * 512], ps[:])
                return go

            def outproj_group(mt, st):
                # full 8-chunk output projection for one 128-row m-tile
                # (used for all but the last s-tile)
                def go():
                    ps = pp.tile([128, 512], F32, tag="proj",
                                 name=f"pso{mt}_{st}")
                    for i in range(n_dch):
                        g, p = i // n_pairs, i % n_pairs
                        c = g * n_pairs + p
                        nc.tensor.matmul(
                            ps[:],
                            lhsT=wo_sb[:, c, mt * 128:(mt + 1) * 128],
                            rhs=af_sb[:, c, st * 512:(st + 1) * 512],
                            start=(i == 0), stop=(i == n_dch - 1))
                    yc = ycp.tile([128, 512], F32, tag="yc", name=f"yc{mt}_{st}")
                    nc.vector.tensor_copy(yc[:], ps[:])
                    nc.sync.dma_start(
                        out=yT_v[:, mt, st * 512:(st + 1) * 512], in_=yc[:])
                return go

            def outproj_last_unit(mt, pair):
                # last s-tile: per-pair partial (2 chunks) accumulated into
                # yacc (SBUF f32); DMA fires after the pair-3 add
                def go():
                    st = nst - 1
                    ps = pp.tile([128, 512], F32, tag="proj",
                                 name=f"psl{mt}_{pair}")
                    for i, g in enumerate((0, 1)):
                        c = g * n_pairs + pair
                        nc.tensor.matmul(
                            ps[:],
                            lhsT=wo_sb[:, c, mt * 128:(mt + 1) * 128],
                            rhs=af_sb[:, c, st * 512:(st + 1) * 512],
                            start=(i == 0), stop=(i == 1))
                    if pair == 0:
                        nc.vector.tensor_copy(yacc[:, mt, :], ps[:])
                    else:
                        nc.vector.tensor_add(yacc[:, mt, :], yacc[:, mt, :],
                                             ps[:])
                    if pair == n_pairs - 1:
                        nc.sync.dma_start(
                            out=yT_v[:, mt, st * 512:(st + 1) * 512],
                            in_=yacc[:, mt, :])
                return go

            def warm_group():
                def go():
                    wps = ps_s.tile([128, 2, 512], F32, tag="sc",
                                    name="warmf")
                    for wj in range(4):
                        nc.tensor.matmul(
                            wps[:, wj % 2, :],
                            lhsT=warm[:, 0:128], rhs=warm[:],
                            start=True, stop=True)
                return go

            def proj_groups_for_st(st):
                gs = []
                for tt in range(4 * st, 4 * st + 4):
                    gs.append(vproj_group(tt))
                for pair in range(n_pairs):
                    for which in range(2):
                        gs.append(qkproj_group(pair, st, which))
                return gs

            # ---- attention for one (pair, st), software-pipelined ----
            def attention(pair, st, filler, stage, pace):
                ntt = 4 * st + 4
                av0 = ps_av.tile([65, 512], F32, tag="av",
                                 name=f"av0_{pair}_{st}")
                av1 = ps_av.tile([65, 512], F32, tag="av",
                                 name=f"av1_{pair}_{st}")
                av = [av0, av1]
                pts = {}

                def scores_and_exp(tt):
                    kk = tt - 4 * st
                    f0 = kk * 128 if kk > 0 else 0
                    ps = ps_s.tile([128, 2, 512], F32, tag="sc",
                                   name=f"sc{pair}_{st}_{tt}")
                    for h in range(2):
                        lo = h * 64
                        nc.tensor.matmul(
                            ps[:, h, f0:512],
                            lhsT=k_sb[lo:lo + 64, pair,
                                      tt * 128:(tt + 1) * 128],
                            rhs=q_sb[lo:lo + 64, pair,
                                     st * 512 + f0:(st + 1) * 512],
                            start=True, stop=True)
                    pt = p_pool.tile([128, 2, 512], DT, tag="pt",
                                     name=f"pt{pair}_{st}_{tt}")
                    if kk < 0:
                        nc.scalar.activation(pt[:], ps[:], EXP, scale=0.125)
                    else:
                        # diagonal: exp the valid cols, triangular mask on
                        # the boundary 128-col block
                        nc.scalar.activation(
                            pt[:, :, f0:512],
                            ps[:, :, f0:512], EXP, scale=0.125)
                        for h in range(2):
                            nc.vector.tensor_mul(
                                pt[:, h, kk * 128:(kk + 1) * 128],
                                pt[:, h, kk * 128:(kk + 1) * 128],
                                m_sb[:])
                    pts[tt] = pt

                def pv(tt):
                    pt = pts.pop(tt)
                    kk = tt - 4 * st
                    f0 = kk * 128 if kk > 0 else 0
                    for h in range(2):
                        nc.tensor.matmul(
                            av[h][:, f0:512],
                            lhsT=v_sb[:, tt, 2 * pair + h, :],
                            rhs=pt[:, h, f0:512],
                            start=(tt == 0), stop=(tt == ntt - 1))

                for tt in range(ntt + 1):
                    if tt < ntt:
                        scores_and_exp(tt)
                    if tt > 0:
                        pv(tt - 1)
                    pace["done"] += 1
                    owed = (pace["pops"] * pace["done"]) // pace["total"] \
                        - pace["popped"]
                    while filler and owed > 0:
                        filler.pop(0)()
                        pace["popped"] += 1
                        owed -= 1

                # normalize: a = av[0:64] * (1/denom); denom row (psum
                # partition 64) -> sbuf -> DMA to partition 0 (the custom-DVE
                # recip and gpsimd broadcast only read partition 0 correctly)
                den = nrm.tile([65, 2, 512], DT, tag="den",
                               name=f"den{pair}_{st}")
                for h in range(2):
                    nc.vector.tensor_copy(den[:, h, :], av[h][:])
                den0 = nrm.tile([1, 2, 512], F32, tag="den0",
                                name=f"den0_{pair}_{st}")
                nc.gpsimd.dma_start(out=den0[:], in_=den[64:65, :, :])
                r = nrm.tile([1, 2, 512], F32, tag="r", name=f"r{pair}_{st}")
                nc.vector.reciprocal_approx_fast(r[:], den0[:])
                bb = nrm.tile([64, 2, 512], F32, tag="b", name=f"bb{pair}_{st}")
                nc.gpsimd.partition_broadcast(bb[:], r[:])
                for h in range(2):
                    nc.vector.tensor_mul(
                        stage[:, h, :],
                        den[0:64, h, :], bb[:, h, :])

            def fire_ag(st, pair, stage):
                # stage this pair's normalized attention columns; for all but
                # the last s-tile one 512KB AllGather per s-tile fires after
                # pair 3 (better cc efficiency); the last s-tile uses per-pair
                # 128KB AllGathers so the tail only waits on pair 3's
                last = st == nst - 1
                if not last:
                    nc.gpsimd.dma_start(
                        out=ag_in[st, pair], in_=stage[:])
                    if pair == n_pairs - 1:
                        nc.gpsimd.collective_compute(
                            "AllGather",
                            mybir.AluOpType.bypass,
                            replica_groups=REPLICA_GROUPS,
                            ins=[ag_in[st].opt()],
                            outs=[ag_out[st].opt()],
                        )
                        for g in range(2):
                            nc.sync.dma_start(
                                out=af_sb[:, g * n_pairs:(g + 1) * n_pairs,
                                          st * 512:(st + 1) * 512],
                                in_=ag_out[st, g].rearrange(
                                    "q p h s -> (p h) q s"))
                else:
                    nc.gpsimd.dma_start(
                        out=ag_in3[pair], in_=stage[:])
                    nc.gpsimd.collective_compute(
                        "AllGather",
                        mybir.AluOpType.bypass,
                        replica_groups=REPLICA_GROUPS,
                        ins=[ag_in3[pair].opt()],
                        outs=[ag_out3[pair].opt()],
                    )
                    nc.sync.dma_start(
                        out=af_sb[:, pair::n_pairs,
                                  st * 512:(st + 1) * 512],
                        in_=ag_out3[pair].rearrange(
                            "g p h s -> (p h) g s"))

            # ---------------- main s-tile-outer schedule ----------------
            pending = proj_groups_for_st(0)
            while pending:
                pending.pop(0)()
            for st in range(nst):
                filler = []
                if st + 1 < nst:
                    filler += proj_groups_for_st(st + 1)
                if st >= 1 and st - 1 < nst - 1:
                    for mt in range(4):
                        filler.append(outproj_group(mt, st - 1))

                last = st == nst - 1
                total_iters = n_pairs * (4 * st + 5)
                pace = {"total": total_iters, "done": 0,
                        "pops": len(filler), "popped": 0}
                for pair in range(n_pairs):
                    stage = stg.tile([64, 2, 512], DT, tag="stage",
                                     name=f"stage{st}_{pair}")
                    attention(pair, st, filler, stage, pace)
                    fire_ag(st, pair, stage)
                    if last and pair >= 2:
                        # append the (pair-2)'th per-pair output-projection
                        # units once that pair's AllGather has had time to
                        # land (two pairs of attention later)
                        for mt in range(4):
                            filler.append(outproj_last_unit(mt, pair - 2))
                while filler:
                    filler.pop(0)()
                if last:
                    # drain: pair 2's units, warm filler to bridge the
                    # final AllGather latency, then pair 3's units
                    for mt in range(4):
                        outproj_last_unit(mt, 2)()
                    warm_group()()
                    for mt in range(4):
                        outproj_last_unit(mt, 3)()
    nc.compile()
    return nc


def _make_mask128():
    p = np.arange(128)[:, None]
    f = np.arange(128)[None, :]
    return (p <= f).astype(BF16)


_NC_CACHE = {}


def _get_nc(seq=S, n_pairs=N_PAIRS):
    key = (seq, n_pairs)
    if key not in _NC_CACHE:
        _NC_CACHE[key] = build_nc(seq, n_pairs)
    return _NC_CACHE[key]


def make_in_maps(x, w_qkv, w_o):
    masks = _make_mask128()
    in_maps = []
    for c in range(N_CORES):
        b, hg = c // 2, c % 2
        heads = slice(hg * 8, hg * 8 + 8)
        # xT: [p, st, c, s'] with m = c*128+p, s = st*512+s'
        xTc = x[b].T.reshape(8, 128, 4, 512).transpose(1, 2, 0, 3)
        # weights: [p, c, n] with contraction row m = c*128+p
        def wlay(w2d):  # w2d: [DM rows m, n cols]
            return np.ascontiguousarray(
                w2d.reshape(8, 128, -1).transpose(1, 0, 2)).astype(BF16)

        def wlay_kh(w2d):
            # within-chunk rows k-major (p' = k*2 + h) to match the af
            # AllGather layout
            a = w2d.reshape(8, 2, 64, -1).transpose(2, 1, 0, 3)
            return np.ascontiguousarray(
                a.reshape(128, 8, -1)).astype(BF16)
        in_maps.append({
            "xT": np.ascontiguousarray(xTc).astype(BF16),
            "wq": wlay(w_qkv[0, heads].reshape(512, DM).T),
            "wk": wlay(w_qkv[1, heads].reshape(512, DM).T),
            "wv": wlay(w_qkv[2, heads].reshape(512, DM).T),
            "wo": wlay_kh(w_o[hg * 512:(hg + 1) * 512, :].T),
            "mask128": masks,
        })
    return in_maps


def kernel(x, w_qkv, w_o):
    x = np.asarray(x, dtype=np.float32)
    w_qkv = np.asarray(w_qkv, dtype=np.float32)
    w_o = np.asarray(w_o, dtype=np.float32)

    nc = _get_nc()
    in_maps = make_in_maps(x, w_qkv, w_o)
    res = run_bass_kernel_spmd(nc, in_maps, list(range(N_CORES)), trace=False)

    y = np.empty((B, S, DM), dtype=np.float32)
    for c in range(N_CORES):
        b, hg = c // 2, c % 2
        y[b, :, hg * 512:(hg + 1) * 512] = res.results[c]["yT"].T
    return y


# revision 14
# speedup vs baseline: 1.0479x; 1.0479x over previous
"""Causal multi-head attention on 8 Trainium2 NeuronCores.

Sharding: core c handles batch b = c//2 and head-half hg = c%2 (8 of 16
heads, as 4 pairs). Per core: QKV projection (bf16 matmuls, f32 PSUM),
flash-style causal attention in transposed layout (scores_T[t, s], softmax
denominator via a ones-column at v-col 0), per-pair pairwise AllGather of
the normalized attention outputs, and a column-parallel output projection
(w_o columns sharded host-side per core parity). Host reassembles y from
the per-core [m_half, s] transposed outputs.

Loop structure: s-tiles outer; each s-tile's attention is interleaved with
the next s-tile's QKV-projection matmuls and the previous s-tile's output
projection so TensorE stays dense while ScalarE runs the exps; each pair's
AllGather fires right after its normalize so transfers hide under later
pairs' attention. The last s-tile's output projection accumulates per-pair
into an SBUF f32 tile so only 2 chunks x 4 mt matmuls remain after the
final AllGather lands.
"""
import sys

sys.path.insert(0, "/opt/trn_rl_repo")

import numpy as np
import ml_dtypes

import concourse.bass as bass
import concourse.mybir as mybir
import concourse.tile as tile
from concourse import bacc
from concourse.bass_utils import run_bass_kernel_spmd

BF16 = ml_dtypes.bfloat16
DT = mybir.dt.bfloat16
F32 = mybir.dt.float32
EXP = mybir.ActivationFunctionType.Exp

B, S, DM, H, DK = 4, 2048, 1024, 16, 64
N_CORES = 8
N_PAIRS = 4          # head pairs per core (8 heads)
N_MCH = DM // 128    # m-chunks of the model dim (contraction for QKV proj)
REPLICA_GROUPS = [[0, 1], [2, 3], [4, 5], [6, 7]]


def build_nc(seq=S, n_pairs=N_PAIRS):
    """Build the SPMD kernel graph. seq must be a multiple of 512."""
    nst = seq // 512          # 512-wide s-tiles
    ntt_all = seq // 128      # 128-wide t-tiles
    nc = bacc.Bacc("TRN2", target_bir_lowering=False, debug=False,
                   num_devices=N_CORES)

    nst_ = seq // 512
    xT = nc.dram_tensor("xT", [128, nst_, N_MCH, 512], DT,
                        kind="ExternalInput")
    wq = nc.dram_tensor("wq", [128, N_MCH, 128 * n_pairs], DT,
                        kind="ExternalInput")
    wk = nc.dram_tensor("wk", [128, N_MCH, 128 * n_pairs], DT,
                        kind="ExternalInput")
    wv = nc.dram_tensor("wv", [128, N_MCH, 128 * n_pairs], DT,
                        kind="ExternalInput")
    wo = nc.dram_tensor("wo", [128, 2 * n_pairs, 512], DT,
                        kind="ExternalInput")
    mask128 = nc.dram_tensor("mask128", [128, 128], DT, kind="ExternalInput")
    yT = nc.dram_tensor("yT", [512, seq], F32, kind="ExternalOutput")

    n_dch = 2 * n_pairs   # d-chunks of 128 in the gathered attention
    hw = 128 * n_pairs    # head-dim columns per core (2*n_pairs heads x 64)

    with tile.TileContext(nc) as tc:
        with (
            tc.tile_pool(name="dram", bufs=1, space="DRAM") as dram,
            tc.tile_pool(name="persist", bufs=1) as persist,
            tc.tile_pool(name="psum_p", bufs=2, space="PSUM") as pp,
            tc.tile_pool(name="psum_s", bufs=2, space="PSUM") as ps_s,
            tc.tile_pool(name="psum_av", bufs=2, space="PSUM") as ps_av,
            tc.tile_pool(name="pt", bufs=4) as p_pool,
            tc.tile_pool(name="nrm", bufs=2) as nrm,
            tc.tile_pool(name="yc", bufs=2) as ycp,
            tc.tile_pool(name="stg", bufs=3) as stg,
        ):
            # ag staging in the stage tile's own [p, h, s] layout so
            # every hop is a contiguous memcpy; af/wo use a matching
            # k-major (p' = k*2 + h) within-chunk row order (host-side)
            ag_in = dram.tile([nst, n_pairs, 64, 2, 512], DT)
            ag_out = dram.tile([nst, 2, n_pairs, 64, 2, 512], DT)
            ag_in3 = dram.tile([n_pairs, 64, 2, 512], DT)
            ag_out3 = dram.tile([n_pairs, 2, 64, 2, 512], DT)

            q_sb = persist.tile([128, n_pairs, seq], DT, tag="q")
            k_sb = persist.tile([128, n_pairs, seq], DT, tag="k")
            v_sb = persist.tile([128, ntt_all, 2 * n_pairs, 65], DT, tag="v")
            af_sb = persist.tile([128, n_dch, seq], DT, tag="af")
            m_sb = persist.tile([128, 128], DT, tag="m")
            wo_sb = persist.tile([128, n_dch, 512], DT, tag="wo")
            wq_sb = persist.tile([128, N_MCH, hw], DT, tag="wq")
            wk_sb = persist.tile([128, N_MCH, hw], DT, tag="wk")
            wv_sb = persist.tile([128, N_MCH, hw], DT, tag="wv")
            yacc = persist.tile([128, 4, 512], F32, tag="yacc")
            xt = []
            for st in range(nst):
                t = persist.tile([128, N_MCH, 512], DT, tag=f"xt{st}")
                xt.append(t)

            # spread input loads across per-engine DMA queues so the
            # critical tiles (wv, xt0, wq) land concurrently; host supplies
            # layouts that DMA as contiguous multi-KB descriptors. wv and
            # xt0 are split across two queues so the first vproj starts
            # as early as possible
            nc.sync.dma_start(out=wv_sb[:, 0:N_MCH // 2],
                              in_=wv[:, 0:N_MCH // 2])
            nc.scalar.dma_start(out=wv_sb[:, N_MCH // 2:],
                                in_=wv[:, N_MCH // 2:])
            nc.sync.dma_start(out=xt[0][:, 0:N_MCH // 2],
                              in_=xT[:, 0, 0:N_MCH // 2])
            nc.scalar.dma_start(out=xt[0][:, N_MCH // 2:],
                                in_=xT[:, 0, N_MCH // 2:])
            nc.gpsimd.dma_start(out=wq_sb[:], in_=wq[:])
            nc.scalar.dma_start(out=wk_sb[:], in_=wk[:])
            if nst > 1:
                nc.gpsimd.dma_start(out=xt[1][:], in_=xT[:, 1])
            for st in range(2, nst):
                nc.sync.dma_start(out=xt[st][:], in_=xT[:, st])
            nc.gpsimd.dma_start(out=m_sb[:], in_=mask128[:])
            nc.sync.dma_start(out=wo_sb[:], in_=wo[:])
            nc.vector.memset(v_sb[:, :, :, 64], 1.0)
            # PE warm-up during the input-load window: dummy matmuls on a
            # memset tile keep the HAM busy-window open (no DMA dependency)
            warm = persist.tile([128, 512], DT, tag="warm")
            nc.vector.memset(warm[:], 0.0)
            for wi in range(3):
                wps = ps_s.tile([128, 2, 512], F32, tag="sc",
                                name=f"warm{wi}")
                for wj in range(8):
                    nc.tensor.matmul(
                        wps[:, wj % 2, :],
                        lhsT=warm[:, 0:128], rhs=warm[:],
                        start=True, stop=True)

            yT_v = yT[:].rearrange("(t p) s -> p t s", p=128)

            # ---- emission helpers (each returns a closure doing one
            # PE-dense psum-group; used to fill PE during attention) ----
            def vproj_group(tt):
                def go():
                    st, r = tt // 4, tt % 4
                    ps = pp.tile([128, hw], F32, tag="proj", name=f"psv{tt}")
                    for c in range(N_MCH):
                        nc.tensor.matmul(
                            ps[:],
                            lhsT=xt[st][:, c, r * 128:(r + 1) * 128],
                            rhs=wv_sb[:, c, 0:hw],
                            start=(c == 0), stop=(c == N_MCH - 1))
                    nc.vector.tensor_copy(
                        v_sb[:, tt, :, 0:64],
                        ps[:].rearrange("p (h k) -> p h k", k=64))
                return go

            def qkproj_group(pair, st, which):
                def go():
                    w_sb, dst = ((wq_sb, q_sb), (wk_sb, k_sb))[which]
                    ps = pp.tile([128, 512], F32, tag="proj",
                                 name=f"psqk{pair}_{st}_{which}")
                    for c in range(N_MCH):
                        nc.tensor.matmul(
                            ps[:],
                            lhsT=w_sb[:, c, pair * 128:(pair + 1) * 128],
                            rhs=xt[st][:, c, :],
                            start=(c == 0), stop=(c == N_MCH - 1))
                    nc.vector.tensor_copy(
                        dst[:, pair, st * 512:(st + 1) * 512], ps[:])
                return go

            def outproj_group(mt, st):
                # full 8-chunk output projection for one 128-row m-tile
                # (used for all but the last s-tile)
                def go():
                    ps = pp.tile([128, 512], F32, tag="proj",
                                 name=f"pso{mt}_{st}")
                    for i in range(n_dch):
                        g, p = i // n_pairs, i % n_pairs
                        c = g * n_pairs + p
                        nc.tensor.matmul(
                            ps[:],
                            lhsT=wo_sb[:, c, mt * 128:(mt + 1) * 128],
                            rhs=af_sb[:, c, st * 512:(st + 1) * 512],
                            start=(i == 0), stop=(i == n_dch - 1))
                    yc = ycp.tile([128, 512], F32, tag="yc", name=f"yc{mt}_{st}")
                    nc.vector.tensor_copy(yc[:], ps[:])
                    nc.sync.dma_start(
                        out=yT_v[:, mt, st * 512:(st + 1) * 512], in_=yc[:])
                return go

            def outproj_last_unit(mt, pair):
                # last s-tile: per-pair partial (2 chunks) accumulated into
                # yacc (SBUF f32); DMA fires after the pair-3 add
                def go():
                    st = nst - 1
                    ps = pp.tile([128, 512], F32, tag="proj",
                                 name=f"psl{mt}_{pair}")
                    for i, g in enumerate((0, 1)):
                        c = g * n_pairs + pair
                        nc.tensor.matmul(
                            ps[:],
                            lhsT=wo_sb[:, c, mt * 128:(mt + 1) * 128],
                            rhs=af_sb[:, c, st * 512:(st + 1) * 512],
                            start=(i == 0), stop=(i == 1))
                    if pair == 0:
                        nc.vector.tensor_copy(yacc[:, mt, :], ps[:])
                    else:
                        nc.vector.tensor_add(yacc[:, mt, :], yacc[:, mt, :],
                                             ps[:])
                    if pair == n_pairs - 1:
                        nc.sync.dma_start(
                            out=yT_v[:, mt, st * 512:(st + 1) * 512],
                            in_=yacc[:, mt, :])
                return go

            def warm_group():
                def go():
                    wps = ps_s.tile([128, 2, 512], F32, tag="sc",
                                    name="warmf")
                    for wj in range(4):
                        nc.tensor.matmul(
                            wps[:, wj % 2, :],
                            lhsT=warm[:, 0:128], rhs=warm[:],
                            start=True, stop=True)
                return go

            def proj_groups_for_st(st):
                gs = []
                for tt in range(4 * st, 4 * st + 4):
                    gs.append(vproj_group(tt))
                for pair in range(n_pairs):
                    for which in range(2):
                        gs.append(qkproj_group(pair, st, which))
                return gs

            # ---- attention for one (pair, st), software-pipelined ----
            def attention(pair, st, filler, stage, pace):
                ntt = 4 * st + 4
                av0 = ps_av.tile([65, 512], F32, tag="av",
                                 name=f"av0_{pair}_{st}")
                av1 = ps_av.tile([65, 512], F32, tag="av",
                                 name=f"av1_{pair}_{st}")
                av = [av0, av1]
                pts = {}

                def scores_and_exp(tt):
                    kk = tt - 4 * st
                    f0 = kk * 128 if kk > 0 else 0
                    ps = ps_s.tile([128, 2, 512], F32, tag="sc",
                                   name=f"sc{pair}_{st}_{tt}")
                    for h in range(2):
                        lo = h * 64
                        nc.tensor.matmul(
                            ps[:, h, f0:512],
                            lhsT=k_sb[lo:lo + 64, pair,
                                      tt * 128:(tt + 1) * 128],
                            rhs=q_sb[lo:lo + 64, pair,
                                     st * 512 + f0:(st + 1) * 512],
                            start=True, stop=True)
                    pt = p_pool.tile([128, 2, 512], DT, tag="pt",
                                     name=f"pt{pair}_{st}_{tt}")
                    if kk < 0:
                        nc.scalar.activation(pt[:], ps[:], EXP, scale=0.125)
                    else:
                        # diagonal: exp the valid cols, triangular mask on
                        # the boundary 128-col block
                        nc.scalar.activation(
                            pt[:, :, f0:512],
                            ps[:, :, f0:512], EXP, scale=0.125)
                        for h in range(2):
                            nc.vector.tensor_mul(
                                pt[:, h, kk * 128:(kk + 1) * 128],
                                pt[:, h, kk * 128:(kk + 1) * 128],
                                m_sb[:])
                    pts[tt] = pt

                def pv(tt):
                    pt = pts.pop(tt)
                    kk = tt - 4 * st
                    f0 = kk * 128 if kk > 0 else 0
                    for h in range(2):
                        nc.tensor.matmul(
                            av[h][:, f0:512],
                            lhsT=v_sb[:, tt, 2 * pair + h, :],
                            rhs=pt[:, h, f0:512],
                            start=(tt == 0), stop=(tt == ntt - 1))

                for tt in range(ntt + 1):
                    if tt < ntt:
                        scores_and_exp(tt)
                    if tt > 0:
                        pv(tt - 1)
                    pace["done"] += 1
                    owed = (pace["pops"] * pace["done"]) // pace["total"] \
                        - pace["popped"]
                    while filler and owed > 0:
                        filler.pop(0)()
                        pace["popped"] += 1
                        owed -= 1

                # normalize: a = av[0:64] * (1/denom); denom row (psum
                # partition 64) -> sbuf -> DMA to partition 0 (the custom-DVE
                # recip and gpsimd broadcast only read partition 0 correctly)
                den = nrm.tile([65, 2, 512], DT, tag="den",
                               name=f"den{pair}_{st}")
                for h in range(2):
                    nc.vector.tensor_copy(den[:, h, :], av[h][:])
                den0 = nrm.tile([1, 2, 512], F32, tag="den0",
                                name=f"den0_{pair}_{st}")
                nc.gpsimd.dma_start(out=den0[:], in_=den[64:65, :, :])
                r = nrm.tile([1, 2, 512], F32, tag="r", name=f"r{pair}_{st}")
                nc.vector.reciprocal_approx_fast(r[:], den0[:])
                bb = nrm.tile([64, 2, 512], F32, tag="b", name=f"bb{pair}_{st}")
                nc.gpsimd.partition_broadcast(bb[:], r[:])
                for h in range(2):
                    nc.vector.tensor_mul(
                        stage[:, h, :],
                        den[0:64, h, :], bb[:, h, :])

            def fire_ag(st, pair, stage):
                # stage this pair's normalized attention columns; for all but
                # the last s-tile one 512KB AllGather per s-tile fires after
                # pair 3 (better cc efficiency); the last s-tile uses per-pair
                # 128KB AllGathers so the tail only waits on pair 3's
                last = st == nst - 1
                if not last:
                    nc.gpsimd.dma_start(
                        out=ag_in[st, pair], in_=stage[:])
                    if pair == n_pairs - 1:
                        nc.gpsimd.collective_compute(
                            "AllGather",
                            mybir.AluOpType.bypass,
                            replica_groups=REPLICA_GROUPS,
                            ins=[ag_in[st].opt()],
                            outs=[ag_out[st].opt()],
                        )
                        for g in range(2):
                            nc.sync.dma_start(
                                out=af_sb[:, g * n_pairs:(g + 1) * n_pairs,
                                          st * 512:(st + 1) * 512],
                                in_=ag_out[st, g].rearrange(
                                    "q p h s -> (p h) q s"))
                else:
                    nc.gpsimd.dma_start(
                        out=ag_in3[pair], in_=stage[:])
                    nc.gpsimd.collective_compute(
                        "AllGather",
                        mybir.AluOpType.bypass,
                        replica_groups=REPLICA_GROUPS,
                        ins=[ag_in3[pair].opt()],
                        outs=[ag_out3[pair].opt()],
                    )
                    nc.sync.dma_start(
                        out=af_sb[:, pair::n_pairs,
                                  st * 512:(st + 1) * 512],
                        in_=ag_out3[pair].rearrange(
                            "g p h s -> (p h) g s"))

            # ---------------- main s-tile-outer schedule ----------------
            pending = proj_groups_for_st(0)
            while pending:
                pending.pop(0)()
            for st in range(nst):
                filler = []
                if st + 1 < nst:
                    filler += proj_groups_for_st(st + 1)
                if st >= 1 and st - 1 < nst - 1:
                    for mt in range(4):
                        filler.append(outproj_group(mt, st - 1))

                last = st == nst - 1
                total_iters = n_pairs * (4 * st + 5)
                pace = {"total": total_iters, "done": 0,
                        "pops": len(filler), "popped": 0}
                for pair in range(n_pairs):
                    stage = stg.tile([64, 2, 512], DT, tag="stage",
                                     name=f"stage{st}_{pair}")
                    attention(pair, st, filler, stage, pace)
                    fire_ag(st, pair, stage)
                    if last and pair >= 2:
                        # append the (pair-2)'th per-pair output-projection
                        # units once that pair's AllGather has had time to
                        # land (two pairs of attention later)
                        for mt in range(4):
                            filler.append(outproj_last_unit(mt, pair - 2))
                while filler:
                    filler.pop(0)()
                if last:
                    # drain: pair 2's units, warm filler to bridge the
                    # final AllGather latency, then pair 3's units
                    for mt in range(4):
                        outproj_last_unit(mt, 2)()
                    warm_group()()
                    for mt in range(4):
                        outproj_last_unit(mt, 3)()
    nc.compile()
    return nc


def _make_mask128():
    p = np.arange(128)[:, None]
    f = np.arange(128)[None, :]
    return (p <= f).astype(BF16)


_NC_CACHE = {}


def _get_nc(seq=S, n_pairs=N_PAIRS):
    key = (seq, n_pairs)
    if key not in _NC_CACHE:
        _NC_CACHE[key] = build_nc(seq, n_pairs)
    return _NC_CACHE[key]


def make_in_maps(x, w_qkv, w_o):
    masks = _make_mask128()
    in_maps = []
    for c in range(N_CORES):
        b, hg = c // 2, c % 2
        heads = slice(hg * 8, hg * 8 + 8)
        # xT: [p, st, c, s'] with m = c*128+p, s = st*512+s'
        xTc = x[b].T.reshape(8, 128, 4, 512).transpose(1, 2, 0, 3)
        # weights: [p, c, n] with contraction row m = c*128+p
        def wlay(w2d):  # w2d: [DM rows m, n cols]
            return np.ascontiguousarray(
                w2d.reshape(8, 128, -1).transpose(1, 0, 2)).astype(BF16)

        def wlay_kh(w2d):
            # within-chunk rows k-major (p' = k*2 + h) to match the af
            # AllGather layout
            a = w2d.reshape(8, 2, 64, -1).transpose(2, 1, 0, 3)
            return np.ascontiguousarray(
                a.reshape(128, 8, -1)).astype(BF16)
        in_maps.append({
            "xT": np.ascontiguousarray(xTc).astype(BF16),
            "wq": wlay(w_qkv[0, heads].reshape(512, DM).T),
            "wk": wlay(w_qkv[1, heads].reshape(512, DM).T),
            "wv": wlay(w_qkv[2, heads].reshape(512, DM).T),
            "wo": wlay_kh(w_o[hg * 512:(hg + 1) * 512, :].T),
            "mask128": masks,
        })
    return in_maps


def kernel(x, w_qkv, w_o):
    x = np.asarray(x, dtype=np.float32)
    w_qkv = np.asarray(w_qkv, dtype=np.float32)
    w_o = np.asarray(w_o, dtype=np.float32)

    nc = _get_nc()
    in_maps = make_in_maps(x, w_qkv, w_o)
    res = run_bass_kernel_spmd(nc, in_maps, list(range(N_CORES)), trace=False)

    y = np.empty((B, S, DM), dtype=np.float32)
    for c in range(N_CORES):
        b, hg = c // 2, c % 2
        y[b, :, hg * 512:(hg + 1) * 512] = res.results[c]["yT"].T
    return y


# revision 17
# speedup vs baseline: 1.0609x; 1.0124x over previous
"""Causal multi-head attention on 8 Trainium2 NeuronCores.

Sharding: core c handles batch b = c//2 and head-half hg = c%2 (8 of 16
heads, as 4 pairs). Per core: QKV projection (bf16 matmuls, f32 PSUM),
flash-style causal attention in transposed layout (scores_T[t, s], softmax
denominator via a ones-column at v-col 0), per-pair pairwise AllGather of
the normalized attention outputs, and a column-parallel output projection
(w_o columns sharded host-side per core parity). Host reassembles y from
the per-core [m_half, s] transposed outputs.

Loop structure: s-tiles outer; each s-tile's attention is interleaved with
the next s-tile's QKV-projection matmuls and the previous s-tile's output
projection so TensorE stays dense while ScalarE runs the exps; each pair's
AllGather fires right after its normalize so transfers hide under later
pairs' attention. The last s-tile's output projection accumulates per-pair
into an SBUF f32 tile so only 2 chunks x 4 mt matmuls remain after the
final AllGather lands.
"""
import sys

sys.path.insert(0, "/opt/trn_rl_repo")

import numpy as np
import ml_dtypes

import concourse.bass as bass
import concourse.mybir as mybir
import concourse.tile as tile
from concourse import bacc
from concourse.bass_utils import run_bass_kernel_spmd

BF16 = ml_dtypes.bfloat16
DT = mybir.dt.bfloat16
F32 = mybir.dt.float32
EXP = mybir.ActivationFunctionType.Exp

B, S, DM, H, DK = 4, 2048, 1024, 16, 64
N_CORES = 8
N_PAIRS = 4          # head pairs per core (8 heads)
N_MCH = DM // 128    # m-chunks of the model dim (contraction for QKV proj)
REPLICA_GROUPS = [[0, 1], [2, 3], [4, 5], [6, 7]]


def build_nc(seq=S, n_pairs=N_PAIRS):
    """Build the SPMD kernel graph. seq must be a multiple of 512."""
    nst = seq // 512          # 512-wide s-tiles
    ntt_all = seq // 128      # 128-wide t-tiles
    nc = bacc.Bacc("TRN2", target_bir_lowering=False, debug=False,
                   num_devices=N_CORES)

    nst_ = seq // 512
    xT = nc.dram_tensor("xT", [128, nst_, N_MCH, 512], DT,
                        kind="ExternalInput")
    wq = nc.dram_tensor("wq", [128, N_MCH, 128 * n_pairs], DT,
                        kind="ExternalInput")
    wk = nc.dram_tensor("wk", [128, N_MCH, 128 * n_pairs], DT,
                        kind="ExternalInput")
    wv = nc.dram_tensor("wv", [128, N_MCH, 128 * n_pairs], DT,
                        kind="ExternalInput")
    wo = nc.dram_tensor("wo", [128, 2 * n_pairs, 512], DT,
                        kind="ExternalInput")
    mask128 = nc.dram_tensor("mask128", [128, 128], DT, kind="ExternalInput")
    yT = nc.dram_tensor("yT", [512, seq], F32, kind="ExternalOutput")

    n_dch = 2 * n_pairs   # d-chunks of 128 in the gathered attention
    hw = 128 * n_pairs    # head-dim columns per core (2*n_pairs heads x 64)

    with tile.TileContext(nc) as tc:
        with (
            tc.tile_pool(name="dram", bufs=1, space="DRAM") as dram,
            tc.tile_pool(name="persist", bufs=1) as persist,
            tc.tile_pool(name="psum_p", bufs=2, space="PSUM") as pp,
            tc.tile_pool(name="psum_s", bufs=2, space="PSUM") as ps_s,
            tc.tile_pool(name="psum_av", bufs=2, space="PSUM") as ps_av,
            tc.tile_pool(name="pt", bufs=4) as p_pool,
            tc.tile_pool(name="nrm", bufs=2) as nrm,
            tc.tile_pool(name="yc", bufs=2) as ycp,
            tc.tile_pool(name="stg", bufs=3) as stg,
        ):
            # ag staging in the stage tile's own [p, h, s] layout so
            # every hop is a contiguous memcpy; af/wo use a matching
            # k-major (p' = k*2 + h) within-chunk row order (host-side)
            ag_in = dram.tile([nst, n_pairs, 64, 2, 512], DT)
            ag_out = dram.tile([nst, 2, n_pairs, 64, 2, 512], DT)
            ag_in3 = dram.tile([n_pairs, 64, 2, 512], DT)
            ag_out3 = dram.tile([n_pairs, 2, 64, 2, 512], DT)

            q_sb = persist.tile([128, n_pairs, seq], DT, tag="q")
            k_sb = persist.tile([128, n_pairs, seq], DT, tag="k")
            v_sb = persist.tile([128, ntt_all, 2 * n_pairs, 65], DT, tag="v")
            af_sb = persist.tile([128, n_dch, seq], DT, tag="af")
            m_sb = persist.tile([128, 128], DT, tag="m")
            wo_sb = persist.tile([128, n_dch, 512], DT, tag="wo")
            wq_sb = persist.tile([128, N_MCH, hw], DT, tag="wq")
            wk_sb = persist.tile([128, N_MCH, hw], DT, tag="wk")
            wv_sb = persist.tile([128, N_MCH, hw], DT, tag="wv")
            yacc = persist.tile([128, 4, 512], F32, tag="yacc")
            xt = []
            for st in range(nst):
                t = persist.tile([128, N_MCH, 512], DT, tag=f"xt{st}")
                xt.append(t)

            # first-needed inputs (xt0, wv, wq, wk) are split in thirds
            # across the three DMA queues so each lands as early as the
            # aggregate load bandwidth allows; host supplies layouts that
            # DMA as contiguous multi-KB descriptors
            thirds = [(0, 3), (3, 6), (6, N_MCH)]
            queues = [nc.sync, nc.scalar, nc.gpsimd]
            for dst, srct in ((xt[0], xT[:, 0]), (wv_sb, wv[:]),
                              (wq_sb, wq[:]), (wk_sb, wk[:])):
                for q, (a, b) in zip(queues, thirds):
                    q.dma_start(out=dst[:, a:b], in_=srct[:, a:b])
            if nst > 1:
                nc.gpsimd.dma_start(out=xt[1][:], in_=xT[:, 1])
            for st in range(2, nst):
                nc.sync.dma_start(out=xt[st][:], in_=xT[:, st])
            nc.gpsimd.dma_start(out=m_sb[:], in_=mask128[:])
            nc.sync.dma_start(out=wo_sb[:], in_=wo[:])
            nc.vector.memset(v_sb[:, :, :, 64], 1.0)
            # PE warm-up during the input-load window: dummy matmuls on a
            # memset tile keep the HAM busy-window open (no DMA dependency)
            warm = persist.tile([128, 512], DT, tag="warm")
            nc.vector.memset(warm[:], 0.0)
            for wi in range(4):
                wps = ps_s.tile([128, 2, 512], F32, tag="sc",
                                name=f"warm{wi}")
                for wj in range(8):
                    nc.tensor.matmul(
                        wps[:, wj % 2, :],
                        lhsT=warm[:, 0:128], rhs=warm[:],
                        start=True, stop=True)

            yT_v = yT[:].rearrange("(t p) s -> p t s", p=128)

            # ---- emission helpers (each returns a closure doing one
            # PE-dense psum-group; used to fill PE during attention) ----
            def vproj_group(tt):
                def go():
                    st, r = tt // 4, tt % 4
                    ps = pp.tile([128, hw], F32, tag="proj", name=f"psv{tt}")
                    for c in range(N_MCH):
                        nc.tensor.matmul(
                            ps[:],
                            lhsT=xt[st][:, c, r * 128:(r + 1) * 128],
                            rhs=wv_sb[:, c, 0:hw],
                            start=(c == 0), stop=(c == N_MCH - 1))
                    nc.vector.tensor_copy(
                        v_sb[:, tt, :, 0:64],
                        ps[:].rearrange("p (h k) -> p h k", k=64))
                return go

            def qkproj_group(pair, st, which):
                def go():
                    w_sb, dst = ((wq_sb, q_sb), (wk_sb, k_sb))[which]
                    ps = pp.tile([128, 512], F32, tag="proj",
                                 name=f"psqk{pair}_{st}_{which}")
                    for c in range(N_MCH):
                        nc.tensor.matmul(
                            ps[:],
                            lhsT=w_sb[:, c, pair * 128:(pair + 1) * 128],
                            rhs=xt[st][:, c, :],
                            start=(c == 0), stop=(c == N_MCH - 1))
                    nc.vector.tensor_copy(
                        dst[:, pair, st * 512:(st + 1) * 512], ps[:])
                return go

            def outproj_group(mt, st):
                # full 8-chunk output projection for one 128-row m-tile
                # (used for all but the last s-tile)
                def go():
                    ps = pp.tile([128, 512], F32, tag="proj",
                                 name=f"pso{mt}_{st}")
                    for i in range(n_dch):
                        g, p = i // n_pairs, i % n_pairs
                        c = g * n_pairs + p
                        nc.tensor.matmul(
                            ps[:],
                            lhsT=wo_sb[:, c, mt * 128:(mt + 1) * 128],
                            rhs=af_sb[:, c, st * 512:(st + 1) * 512],
                            start=(i == 0), stop=(i == n_dch - 1))
                    yc = ycp.tile([128, 512], F32, tag="yc", name=f"yc{mt}_{st}")
                    nc.vector.tensor_copy(yc[:], ps[:])
                    nc.sync.dma_start(
                        out=yT_v[:, mt, st * 512:(st + 1) * 512], in_=yc[:])
                return go

            ps_y = [None, None]

            def outproj_last_unit(mt, pair, first=False, final=False):
                # last s-tile: per-pair partials (2 chunks each) accumulate
                # directly in PSUM (borrowing the idle scores-pool banks);
                # the final pair's units stop the group, then ScalarE copies
                # out and the result DMAs to yT
                def go():
                    st = nst - 1
                    if first and mt == 0:
                        ps_y[0] = ps_s.tile([128, 2, 512], F32, tag="sc",
                                            name="yps0")
                        ps_y[1] = ps_s.tile([128, 2, 512], F32, tag="sc",
                                            name="yps1")
                    ps = ps_y[mt // 2]
                    for i, g in enumerate((0, 1)):
                        c = g * n_pairs + pair
                        nc.tensor.matmul(
                            ps[:, mt % 2, :],
                            lhsT=wo_sb[:, c, mt * 128:(mt + 1) * 128],
                            rhs=af_sb[:, c, st * 512:(st + 1) * 512],
                            start=(first and i == 0),
                            stop=(final and i == 1))
                    if final:
                        yc = ycp.tile([128, 512], F32, tag="yc",
                                      name=f"ycl{mt}")
                        nc.scalar.copy(yc[:], ps[:, mt % 2, :])
                        nc.sync.dma_start(
                            out=yT_v[:, mt, st * 512:(st + 1) * 512],
                            in_=yc[:])
                return go

            def warm_group():
                def go():
                    wps = ps_s.tile([128, 2, 512], F32, tag="sc",
                                    name="warmf")
                    for wj in range(4):
                        nc.tensor.matmul(
                            wps[:, wj % 2, :],
                            lhsT=warm[:, 0:128], rhs=warm[:],
                            start=True, stop=True)
                return go

            def proj_groups_for_st(st):
                gs = []
                for tt in range(4 * st, 4 * st + 4):
                    gs.append(vproj_group(tt))
                for pair in range(n_pairs):
                    for which in range(2):
                        gs.append(qkproj_group(pair, st, which))
                return gs

            # ---- attention for one (pair, st), software-pipelined ----
            def attention(pair, st, filler, stage, pace):
                ntt = 4 * st + 4
                av0 = ps_av.tile([65, 512], F32, tag="av",
                                 name=f"av0_{pair}_{st}")
                av1 = ps_av.tile([65, 512], F32, tag="av",
                                 name=f"av1_{pair}_{st}")
                av = [av0, av1]
                pts = {}

                def scores_and_exp(tt):
                    kk = tt - 4 * st
                    f0 = kk * 128 if kk > 0 else 0
                    ps = ps_s.tile([128, 2, 512], F32, tag="sc",
                                   name=f"sc{pair}_{st}_{tt}")
                    for h in range(2):
                        lo = h * 64
                        nc.tensor.matmul(
                            ps[:, h, f0:512],
                            lhsT=k_sb[lo:lo + 64, pair,
                                      tt * 128:(tt + 1) * 128],
                            rhs=q_sb[lo:lo + 64, pair,
                                     st * 512 + f0:(st + 1) * 512],
                            start=True, stop=True)
                    pt = p_pool.tile([128, 2, 512], DT, tag="pt",
                                     name=f"pt{pair}_{st}_{tt}")
                    if kk < 0:
                        nc.scalar.activation(pt[:], ps[:], EXP, scale=0.125)
                    else:
                        # diagonal: exp the valid cols, triangular mask on
                        # the boundary 128-col block
                        nc.scalar.activation(
                            pt[:, :, f0:512],
                            ps[:, :, f0:512], EXP, scale=0.125)
                        for h in range(2):
                            nc.vector.tensor_mul(
                                pt[:, h, kk * 128:(kk + 1) * 128],
                                pt[:, h, kk * 128:(kk + 1) * 128],
                                m_sb[:])
                    pts[tt] = pt

                def pv(tt):
                    pt = pts.pop(tt)
                    kk = tt - 4 * st
                    f0 = kk * 128 if kk > 0 else 0
                    for h in range(2):
                        nc.tensor.matmul(
                            av[h][:, f0:512],
                            lhsT=v_sb[:, tt, 2 * pair + h, :],
                            rhs=pt[:, h, f0:512],
                            start=(tt == 0), stop=(tt == ntt - 1))

                for tt in range(ntt + 1):
                    if tt < ntt:
                        scores_and_exp(tt)
                    if tt > 0:
                        pv(tt - 1)
                    pace["done"] += 1
                    owed = (pace["pops"] * pace["done"]) // pace["total"] \
                        - pace["popped"]
                    while filler and owed > 0:
                        filler.pop(0)()
                        pace["popped"] += 1
                        owed -= 1

                # normalize: a = av[0:64] * (1/denom); denom row (psum
                # partition 64) -> sbuf -> DMA to partition 0 (the custom-DVE
                # recip and gpsimd broadcast only read partition 0 correctly).
                # The last pair of the last s-tile runs its copies + DMA on
                # the (by then idle) ScalarE queue so its AllGather fires as
                # early as possible
                crit = pair == n_pairs - 1 and st == nst - 1
                den = nrm.tile([65, 2, 512], F32 if crit else DT, tag="den",
                               name=f"den{pair}_{st}")
                for h in range(2):
                    if crit:
                        nc.scalar.copy(den[:, h, :], av[h][:])
                    else:
                        nc.vector.tensor_copy(den[:, h, :], av[h][:])
                den0 = nrm.tile([1, 2, 512], F32, tag="den0",
                                name=f"den0_{pair}_{st}")
                (nc.scalar if crit else nc.gpsimd).dma_start(
                    out=den0[:], in_=den[64:65, :, :])
                r = nrm.tile([1, 2, 512], F32, tag="r", name=f"r{pair}_{st}")
                nc.vector.reciprocal_approx_fast(r[:], den0[:])
                bb = nrm.tile([64, 2, 512], F32, tag="b", name=f"bb{pair}_{st}")
                nc.gpsimd.partition_broadcast(bb[:], r[:])
                for h in range(2):
                    nc.vector.tensor_mul(
                        stage[:, h, :],
                        den[0:64, h, :], bb[:, h, :])

            def fire_ag(st, pair, stage):
                # stage this pair's normalized attention columns; for all but
                # the last s-tile one 512KB AllGather per s-tile fires after
                # pair 3 (better cc efficiency); the last s-tile uses per-pair
                # 128KB AllGathers so the tail only waits on pair 3's
                last = st == nst - 1
                if not last:
                    nc.gpsimd.dma_start(
                        out=ag_in[st, pair], in_=stage[:])
                    if pair == n_pairs - 1:
                        nc.gpsimd.collective_compute(
                            "AllGather",
                            mybir.AluOpType.bypass,
                            replica_groups=REPLICA_GROUPS,
                            ins=[ag_in[st].opt()],
                            outs=[ag_out[st].opt()],
                        )
                        for g in range(2):
                            nc.sync.dma_start(
                                out=af_sb[:, g * n_pairs:(g + 1) * n_pairs,
                                          st * 512:(st + 1) * 512],
                                in_=ag_out[st, g].rearrange(
                                    "q p h s -> (p h) q s"))
                else:
                    nc.gpsimd.dma_start(
                        out=ag_in3[pair], in_=stage[:])
                    nc.gpsimd.collective_compute(
                        "AllGather",
                        mybir.AluOpType.bypass,
                        replica_groups=REPLICA_GROUPS,
                        ins=[ag_in3[pair].opt()],
                        outs=[ag_out3[pair].opt()],
                    )
                    nc.sync.dma_start(
                        out=af_sb[:, pair::n_pairs,
                                  st * 512:(st + 1) * 512],
                        in_=ag_out3[pair].rearrange(
                            "g p h s -> (p h) g s"))

            # ---------------- main s-tile-outer schedule ----------------
            pending = proj_groups_for_st(0)
            while pending:
                pending.pop(0)()
            for st in range(nst):
                filler = []
                last = st == nst - 1
                if st + 1 < nst:
                    filler += proj_groups_for_st(st + 1)
                held = []
                if st >= 1 and st - 1 < nst - 1:
                    # for the last window hold back mt 2,3 of the previous
                    # s-tile's output projection to bridge the tail drain
                    for mt in range(4 if not last else 2):
                        filler.append(outproj_group(mt, st - 1))
                    if last:
                        held = [outproj_group(2, st - 1),
                                outproj_group(3, st - 1)]
                total_iters = n_pairs * (4 * st + 5)
                pace = {"total": total_iters, "done": 0,
                        "pops": len(filler), "popped": 0}
                for pair in range(n_pairs):
                    stage = stg.tile([64, 2, 512], DT, tag="stage",
                                     name=f"stage{st}_{pair}")
                    attention(pair, st, filler, stage, pace)
                    fire_ag(st, pair, stage)
                while filler:
                    filler.pop(0)()
                if last:
                    # drain: pairs 0,1 accumulate while their AllGathers are
                    # long done; held st-2 groups and a warm group bridge the
                    # last AllGathers; pair 3's op completes before pair 2's
                    # (cc-stream completion alternation), so pair 2 closes
                    # the accumulation groups
                    for mt in range(4):
                        outproj_last_unit(mt, 0, first=True)()
                    held[0]()
                    for mt in range(4):
                        outproj_last_unit(mt, 1)()
                    held[1]()
                    warm_group()()
                    for mt in range(4):
                        outproj_last_unit(mt, 3)()
                    for mt in range(4):
                        outproj_last_unit(mt, 2, final=True)()
    nc.compile()
    return nc


def _make_mask128():
    p = np.arange(128)[:, None]
    f = np.arange(128)[None, :]
    return (p <= f).astype(BF16)


_NC_CACHE = {}


def _get_nc(seq=S, n_pairs=N_PAIRS):
    key = (seq, n_pairs)
    if key not in _NC_CACHE:
        _NC_CACHE[key] = build_nc(seq, n_pairs)
    return _NC_CACHE[key]


def make_in_maps(x, w_qkv, w_o):
    masks = _make_mask128()
    in_maps = []
    for c in range(N_CORES):
        b, hg = c // 2, c % 2
        heads = slice(hg * 8, hg * 8 + 8)
        # xT: [p, st, c, s'] with m = c*128+p, s = st*512+s'
        xTc = x[b].T.reshape(8, 128, 4, 512).transpose(1, 2, 0, 3)
        # weights: [p, c, n] with contraction row m = c*128+p
        def wlay(w2d):  # w2d: [DM rows m, n cols]
            return np.ascontiguousarray(
                w2d.reshape(8, 128, -1).transpose(1, 0, 2)).astype(BF16)

        def wlay_kh(w2d):
            # within-chunk rows k-major (p' = k*2 + h) to match the af
            # AllGather layout
            a = w2d.reshape(8, 2, 64, -1).transpose(2, 1, 0, 3)
            return np.ascontiguousarray(
                a.reshape(128, 8, -1)).astype(BF16)
        in_maps.append({
            "xT": np.ascontiguousarray(xTc).astype(BF16),
            "wq": wlay(w_qkv[0, heads].reshape(512, DM).T),
            "wk": wlay(w_qkv[1, heads].reshape(512, DM).T),
            "wv": wlay(w_qkv[2, heads].reshape(512, DM).T),
            "wo": wlay_kh(w_o[hg * 512:(hg + 1) * 512, :].T),
            "mask128": masks,
        })
    return in_maps


def kernel(x, w_qkv, w_o):
    x = np.asarray(x, dtype=np.float32)
    w_qkv = np.asarray(w_qkv, dtype=np.float32)
    w_o = np.asarray(w_o, dtype=np.float32)

    nc = _get_nc()
    in_maps = make_in_maps(x, w_qkv, w_o)
    res = run_bass_kernel_spmd(nc, in_maps, list(range(N_CORES)), trace=False)

    y = np.empty((B, S, DM), dtype=np.float32)
    for c in range(N_CORES):
        b, hg = c // 2, c % 2
        y[b, :, hg * 512:(hg + 1) * 512] = res.results[c]["yT"].T
    return y
